# revision 4
# baseline (speedup 1.0000x reference)
"""Bidirectional 2-layer LSTM (B=32, T=256, IN=H=512) on 8 Trainium2 cores.

v2 design (8 cores = 2 directions x 4 batch shards, b=8 per core):
  - The per-step recurrent GEMM is LDWEIGHTS-bound on TRN2 (64 [128x128]
    fp16 weight tiles re-loaded every step), so the kernel minimizes
    everything around that stream:
      * xg (input-gate contributions) is accumulated into the SAME PSUM
        region as the recurrent matmuls via one identity matmul per step
        (no DVE adds on the critical path).
      * Only Sigmoid is used: g-gate pre-activations are scaled 2x in the
        weights so tanh(g) = 2*sigmoid(2g)-1; the cell state is kept
        DOUBLED (c2 = 2c) in PSUM so tanh(c) = 2*sigmoid(c2)-1 needs no
        activation-scale switch; h is stored HALVED (h/2), compensated by
        2x in all consumer weights and a final 2x on the host.
      * Gate tile order [f, i, g, o]: one sigmoid over [f,i,g] starts
        while the o-tiles still stream; c2 is written back into the PSUM
        tile so a single sigmoid over [o | c2] feeds the final h update.
      * Post-stream tail = sem + sigmoid([128,64]) + sem + one DVE
        scalar_tensor_tensor + sem.
  - xg GEMM (no recurrence) is interleaved one m-unit per step into the
    PE idle window of the elementwise tail; bias is folded during the
    PSUM->SBUF copy on the (otherwise idle) GPSIMD engine.
  - All state (xg, h history, weights) is SBUF-resident; zero per-step
    DMA. h history is DMA'd out once at the end.
  - Two launches: layer 0 (nkc=4 input chunks) and layer 1 (nkc=8), with
    a host roundtrip to redistribute/reverse h between directions.
"""

import os
import sys

for _p in ("/opt/trn_rl_repo", "/root/.axon_site/_ro/trn_rl_repo"):
    if os.path.isdir(_p) and _p not in sys.path:
        sys.path.insert(0, _p)

import numpy as np

import concourse.bass as bass
import concourse.bacc as bacc
import concourse.tile as tile
import concourse.mybir as mybir
import concourse.bass_utils as bass_utils

NCORES = 8
NSHARD = 4           # batch shards (cores 2s=fwd, 2s+1=bwd for shard s)
B, T, IN, H = 32, 256, 512, 512
BS = B // NSHARD     # batch per core = 8
T = int(os.environ.get("LSTM_T", T))
TSW = 16             # timesteps per xg sweep
NSWEEP = T // TSW
F16 = mybir.dt.float16
F32 = mybir.dt.float32
ADD = mybir.AluOpType.add
MULT = mybir.AluOpType.mult

# gate tile order [f, i, g, o]; OG maps tile-group -> original gate index
# in the reference's [i, f, g, o] column order.
OG = (1, 0, 2, 3)
_PERM = np.concatenate(
    [512 * OG[m // 4] + 128 * (m % 4) + np.arange(128) for m in range(16)]
)
# per-tile weight scale exponent for the tanh-via-sigmoid trick (g tiles
# are tiles 8..11 in [f,i,g,o] order)
_G_TILES = slice(8, 12)

_NC_CACHE = {}


def _build_launch(nkc):
    """One direction-layer for a b=8 shard. Per-core inputs:
      rhs   [nkc, 128, T*8] f16  -- layer-input tiles, cols (t, b)
      wih   [nkc, 16, 128, 128] f16 -- input-weight tiles (perm'd, scaled)
      whh   [4, 16, 128, 128] f16   -- recurrent-weight tiles (perm'd, x2/x4)
      bias  [128, 16] f32           -- bias per (row, tile), g tiles x2
      ident [128, 128] f16          -- identity for PSUM xg accumulate
    Output:
      hout  [T, 128, 32] f16        -- h/2 per step, layout [Hw, (hb, b)]
    """
    nc = bacc.Bacc("TRN2", target_bir_lowering=False, debug=False,
                   enable_asserts=True, num_devices=NCORES)
    rhs_d = nc.dram_tensor("rhs", [nkc, 128, T * BS], F16,
                           kind="ExternalInput")
    wih_d = nc.dram_tensor("wih", [nkc, 16, 128, 128], F16,
                           kind="ExternalInput")
    whh_d = nc.dram_tensor("whh", [4, 16, 128, 128], F16,
                           kind="ExternalInput")
    bias_d = nc.dram_tensor("bias", [128, 16], F32, kind="ExternalInput")
    ident_d = nc.dram_tensor("ident", [128, 128], F16, kind="ExternalInput")
    hout_d = nc.dram_tensor("hout", [T, 128, 4 * BS], F16,
                            kind="ExternalOutput")

    SIG = mybir.ActivationFunctionType.Sigmoid
    HB = 4 * BS          # h free width = 32
    SWC = TSW * BS       # xg sweep columns = 128

    with tile.TileContext(nc) as tc:
        with tc.tile_pool(name="wpool", bufs=1) as wpool:
            wih_sb = wpool.tile([128, nkc * 16 * 128], F16)
            whh_sb = wpool.tile([128, 4 * 16 * 128], F16)
            rhs_sb = wpool.tile([128, nkc * T * BS], F16)
            bias_sb = wpool.tile([128, 16], F32)
            ident_sb = wpool.tile([128, 128], F16)
            xg_buf = wpool.tile([128, T * 16 * BS], F16)
            hout_buf = wpool.tile([128, T * HB], F16)
            hinit = wpool.tile([128, HB], F16)
            z0 = wpool.tile([128, HB], F32)

            nc.sync.dma_start(
                wih_sb[:].rearrange("k (c m j) -> k c m j",
                                    c=nkc, m=16, j=128),
                wih_d.ap().rearrange("c m k j -> k c m j"))
            nc.sync.dma_start(
                whh_sb[:].rearrange("k (c m j) -> k c m j",
                                    c=4, m=16, j=128),
                whh_d.ap().rearrange("c m k j -> k c m j"))
            nc.sync.dma_start(
                rhs_sb[:].rearrange("k (c t) -> k c t", c=nkc, t=T * BS),
                rhs_d.ap().rearrange("c k t -> k c t"))
            nc.sync.dma_start(bias_sb[:], bias_d.ap())
            nc.sync.dma_start(ident_sb[:], ident_d.ap())
            nc.gpsimd.memset(hinit[:], 0.0)
            nc.gpsimd.memset(z0[:], 0.0)

            def wih_t(c, m):
                o = (c * 16 + m) * 128
                return wih_sb[:, o:o + 128]

            def whh_t(c, m):
                o = (c * 16 + m) * 128
                return whh_sb[:, o:o + 128]

            xg_v = xg_buf[:].rearrange("k (t m b) -> k t m b",
                                       t=T, m=16, b=BS)

            with (
                tc.tile_pool(name="xpsum", bufs=2, space="PSUM") as xpsum,
                tc.tile_pool(name="rpsum", bufs=2, space="PSUM") as rpsum,
                tc.tile_pool(name="sfig", bufs=2) as sfigp,
                tc.tile_pool(name="soc", bufs=2) as socp,
                tc.tile_pool(name="dtmp", bufs=2) as dtmp,
            ):
                def emit_xg_unit(s, m):
                    """xg GEMM for sweep s, m-tile m: psum -> xg_buf."""
                    ps = xpsum.tile([128, SWC], F32, name="xps", tag="xps")
                    mv = rhs_sb[:].rearrange(
                        "k (c t) -> k c t", c=nkc, t=T * BS)
                    for c in range(nkc):
                        nc.tensor.matmul(
                            ps[:], wih_t(c, m),
                            mv[:, c, s * SWC:(s + 1) * SWC],
                            start=(c == 0), stop=(c == nkc - 1))
                    dst = xg_v[:, s * TSW:(s + 1) * TSW, m]
                    src = ps[:].rearrange("k (t b) -> k t b", t=TSW, b=BS)
                    nc.vector.tensor_scalar_add(dst, src, bias_sb[:, m:m + 1])

                upfront = min(2, NSWEEP)
                for s in range(upfront):
                    for m in range(16):
                        emit_xg_unit(s, m)
                units = [(s, m) for s in range(upfront, NSWEEP)
                         for m in range(16)]

                c2_prev = None      # PSUM slice of previous step's tile
                h_prev = hinit

                for t in range(T):
                    ps = rpsum.tile([128, 160], F32, name="ps", tag="ps")
                    # xg for all 16 m-tiles of step t -> PSUM via identity
                    nc.tensor.matmul(ps[:, 0:128], ident_sb[:],
                                     xg_buf[:, t * 128:(t + 1) * 128],
                                     start=True, stop=False)
                    # recurrent MMs, tile order [f,i,g,o] (m-major)
                    for m in range(16):
                        for c in range(4):
                            nc.tensor.matmul(
                                ps[:, m * BS:(m + 1) * BS],
                                whh_t(c, m),
                                h_prev[:, BS * c:BS * (c + 1)],
                                start=False, stop=(m == 15 and c == 3))
                    # sigmoid over [f,i,g] while o-tiles stream
                    s_fig = sfigp.tile([128, 96], F16, tag="sfig")
                    nc.scalar.activation(s_fig[:], ps[:, 0:96], SIG)
                    # cell update (c2 = 2c, in PSUM cols 128:160)
                    t1 = dtmp.tile([128, HB], F32, tag="t1")
                    nc.vector.tensor_mul(
                        t1[:], s_fig[:, 0:HB],
                        c2_prev if c2_prev is not None else z0[:])
                    t2p = dtmp.tile([128, HB], F16, tag="t2p")
                    nc.vector.scalar_tensor_tensor(
                        t2p[:], s_fig[:, 2 * HB:3 * HB], -0.5,
                        s_fig[:, HB:2 * HB], ADD, MULT)
                    nc.vector.scalar_tensor_tensor(
                        ps[:, 128:160], t2p[:], 4.0, t1[:], MULT, ADD)
                    # one sigmoid over [o | c2]
                    s_oc = socp.tile([128, 64], F16, tag="soc")
                    nc.scalar.activation(s_oc[:], ps[:, 96:160], SIG)
                    # h/2 = (sig(c2) - 0.5) * sig(o), straight into hout_buf
                    nc.vector.scalar_tensor_tensor(
                        hout_buf[:, t * HB:(t + 1) * HB],
                        s_oc[:, HB:2 * HB], -0.5, s_oc[:, 0:HB], ADD, MULT)
                    c2_prev = ps[:, 128:160]
                    h_prev = hout_buf[:, t * HB:(t + 1) * HB]
                    # fill the PE tail with one future-sweep xg unit
                    if t < len(units):
                        emit_xg_unit(*units[t])

                nc.sync.dma_start(
                    hout_d.ap().rearrange("t k c -> k t c"),
                    hout_buf[:].rearrange("k (t c) -> k t c", t=T, c=HB))

    nc.compile()
    return nc


def _get_nc(nkc):
    if nkc not in _NC_CACHE:
        _NC_CACHE[nkc] = _build_launch(nkc)
    return _NC_CACHE[nkc]


def _prep_w(w, nkc, scale_fio, scale_g):
    """[Din, 2048] -> [nkc, 16, 128, 128] f16 tiles, col-permuted+scaled."""
    wp = np.asarray(w, dtype=np.float32)[:, _PERM].reshape(-1, 16, 128)
    sc = np.full(16, scale_fio, np.float32)
    sc[_G_TILES] = scale_g
    wp = wp * sc[None, :, None]
    return np.ascontiguousarray(
        wp.reshape(nkc, 128, 16, 128).transpose(0, 2, 1, 3)).astype(np.float16)


def _prep_bias(b):
    """[2048] -> [128, 16] f32 (g tiles x2)."""
    bp = np.asarray(b, dtype=np.float32)[_PERM].reshape(16, 128)
    sc = np.ones(16, np.float32)
    sc[_G_TILES] = 2.0
    return np.ascontiguousarray((bp * sc[:, None]).T)


def _prep_rhs_from_x(x_dir):
    """[BS, T, IN] -> [4, 128, T*8] f16 with cols (t, b)."""
    xt = np.asarray(x_dir, dtype=np.float32).transpose(2, 1, 0)  # [IN, T, B]
    return np.ascontiguousarray(
        xt.reshape(4, 128, T * BS)).astype(np.float16)


def _rhs_from_h(h):
    """hout [T, 128, 32] (t, Hw, (hb,b)) -> [4, 128, T*8] chunk tiles."""
    r = h.reshape(T, 128, 4, BS).transpose(2, 1, 0, 3)  # [hb, k, t, b]
    return np.ascontiguousarray(r.reshape(4, 128, T * BS))


def _unpack_h(h, reverse):
    """hout [T, 128, 32] f16 (h/2) -> [BS, T, H] f32 (x2)."""
    a = h.astype(np.float32).reshape(T, 128, 4, BS).transpose(3, 0, 2, 1)
    a = np.ascontiguousarray(a).reshape(BS, T, H) * 2.0
    return a[:, ::-1, :] if reverse else a


_IDENT = np.eye(128, dtype=np.float16)


def _run(nc, in_maps):
    return bass_utils.run_bass_kernel_spmd(
        nc, in_maps, core_ids=list(range(NCORES)), trace=False)


def _maps_layer0(x, w_ih0f, w_hh0f, b0f, w_ih0b, w_hh0b, b0b):
    wf = {"wih": _prep_w(w_ih0f, 4, 1.0, 2.0),
          "whh": _prep_w(w_hh0f, 4, 2.0, 4.0),
          "bias": _prep_bias(b0f), "ident": _IDENT}
    wb = {"wih": _prep_w(w_ih0b, 4, 1.0, 2.0),
          "whh": _prep_w(w_hh0b, 4, 2.0, 4.0),
          "bias": _prep_bias(b0b), "ident": _IDENT}
    maps = []
    for core in range(NCORES):
        s, fwd = core // 2, core % 2 == 0
        xs = x[s * BS:(s + 1) * BS]
        maps.append(dict(rhs=_prep_rhs_from_x(xs if fwd else xs[:, ::-1]),
                         **(wf if fwd else wb)))
    return maps


def _maps_layer1(res1, w_ih1f, w_hh1f, b1f, w_ih1b, w_hh1b, b1b):
    wf = {"wih": _prep_w(w_ih1f, 8, 2.0, 4.0),
          "whh": _prep_w(w_hh1f, 4, 2.0, 4.0),
          "bias": _prep_bias(b1f), "ident": _IDENT}
    wb = {"wih": _prep_w(w_ih1b, 8, 2.0, 4.0),
          "whh": _prep_w(w_hh1b, 4, 2.0, 4.0),
          "bias": _prep_bias(b1b), "ident": _IDENT}
    maps = []
    for core in range(NCORES):
        s, fwd = core // 2, core % 2 == 0
        h0f = res1[2 * s]["hout"]       # canonical time order
        h0b = res1[2 * s + 1]["hout"]   # bwd scan order (reversed time)
        if fwd:
            rhs = np.concatenate(
                [_rhs_from_h(h0f), _rhs_from_h(h0b[::-1])], axis=0)
        else:
            rhs = np.concatenate(
                [_rhs_from_h(h0f[::-1]), _rhs_from_h(h0b)], axis=0)
        maps.append(dict(rhs=rhs, **(wf if fwd else wb)))
    return maps


def kernel(x, w_ih0f, w_hh0f, b0f, w_ih0b, w_hh0b, b0b,
           w_ih1f, w_hh1f, b1f, w_ih1b, w_hh1b, b1b):
    x = np.asarray(x, dtype=np.float32)

    nc1 = _get_nc(4)
    res1 = _run(nc1, _maps_layer0(
        x, w_ih0f, w_hh0f, b0f, w_ih0b, w_hh0b, b0b)).results

    nc2 = _get_nc(8)
    res2 = _run(nc2, _maps_layer1(
        res1, w_ih1f, w_hh1f, b1f, w_ih1b, w_hh1b, b1b)).results

    out = np.empty((B, T, 2 * H), np.float32)
    for s in range(NSHARD):
        out[s * BS:(s + 1) * BS, :, :H] = _unpack_h(res2[2 * s]["hout"], False)
        out[s * BS:(s + 1) * BS, :, H:] = _unpack_h(res2[2 * s + 1]["hout"], True)
    return out
